# revision 8
# baseline (speedup 1.0000x reference)
"""Cox partial-likelihood loss on 8 Trainium2 NeuronCores.

Changes vs kernel.py:
  - DVE/GpSimd tiles emit w_j * (t_i >= t_j) directly via
    tensor_scalar(op0=is_ge, op1=mult, scalar2=w_col); PE stationary for
    these tiles is a constant ones vector (loaded once).
  - A stride-4 subset of j-tiles runs on ScalarE as sign(t_i - t_j)
    (bf16, exact -1/0/+1), accumulated with stationary wh = bf16(w/2)
    columns.  Identity  w*1{t_j<=t_i} = wh*sign + wh  (for t_j != t_i)
    turns into two corrections:
      * + sum_{j in ACT tiles} wh_j   (uniform; folded into the Ln bias)
      * + 0.5*w_i for rows i whose own column lands in an ACT tile
        (sign(0)=0 there); dsel input carries 0.5/0 per row from the host.
    Cross-sample ties inside ACT tiles are half-counted; measured impact
    on the scalar loss is ~1e-5 relative, far below tolerance.
"""

from contextlib import ExitStack

import numpy as np

import concourse.bacc as bacc
import concourse.mybir as mybir
import concourse.tile as tile
from concourse import bass_utils

F32 = mybir.dt.float32
BF16 = mybir.dt.bfloat16
ALU = mybir.AluOpType
AFT = mybir.ActivationFunctionType
AXL = mybir.AxisListType

N = 16384
NCORES = 8
P = 128
EPS = 1e-7
# ScalarE (ACT) j-tile subset: residues {2, 4} mod 5, trimmed at the
# tail (ACT starts ~8us later than DVE due to its table load + exp
# prologue, and the final tiles should be DVE so the PE drains
# immediately).  Expressed as two strided bounded slices so the Whalf
# reduction stays two strided APs.
ACT_MODULUS = 5


def _act_slices(ct: int):
    # (start, stop, step) python slices of j-tile indices handled by ACT
    if ct < 2 * ACT_MODULUS:
        return []
    return [(2, ct - 1, ACT_MODULUS), (4, ct, ACT_MODULUS)]


def _act_set(ct: int):
    s = set()
    for a, b, st in _act_slices(ct):
        s.update(range(a, b, st))
    return s


def _assign_engines(ct: int) -> list[str]:
    # GpSimd is excluded: its tensor ops share SBUF read ports with the
    # DVE and both engines crawl (~32us/tile measured) when concurrent.
    acts = _act_set(ct)
    return ["a" if c in acts else "v" for c in range(ct)]


def build(n: int = N, ncores: int = NCORES):
    ct = n // P
    rows = n // ncores
    chunk = min(512, rows)
    nch = rows // chunk
    ecols = rows // P
    assert rows % P == 0 and rows % chunk == 0 and n % P == 0

    nc = bacc.Bacc("TRN2", target_bir_lowering=False, debug=False)

    t_all = nc.dram_tensor("t_all", [n], F32, kind="ExternalInput")
    r_all = nc.dram_tensor("r_all", [n], F32, kind="ExternalInput")
    # t_blk arrives as three bf16 components with tb_a+tb_b+tb_c == t_blk
    # exactly (lossless bf16x3 encoding of fp32) so the partition
    # broadcast can use fast bf16 K=1 matmuls with fp32 accumulation —
    # fp32 matmuls run as a ~2x slower LOW/HIGH double pass on the PE.
    tb_a = nc.dram_tensor("tb_a", [rows], BF16, kind="ExternalInput")
    tb_b = nc.dram_tensor("tb_b", [rows], BF16, kind="ExternalInput")
    tb_c = nc.dram_tensor("tb_c", [rows], BF16, kind="ExternalInput")
    r_blk = nc.dram_tensor("r_blk", [rows], F32, kind="ExternalInput")
    e_blk = nc.dram_tensor("e_blk", [rows], F32, kind="ExternalInput")
    dsel_b = nc.dram_tensor("dsel_blk", [rows], F32, kind="ExternalInput")
    out_d = nc.dram_tensor("out", [2, 1], F32, kind="ExternalOutput")

    assign = _assign_engines(ct)
    have_act = "a" in assign

    with tile.TileContext(nc) as tc, ExitStack() as ctx:
        const = ctx.enter_context(tc.tile_pool(name="const", bufs=1))
        masks = ctx.enter_context(tc.tile_pool(name="masks", bufs=12))
        psump = ctx.enter_context(tc.tile_pool(name="psum", bufs=1, space="PSUM"))
        ep = ctx.enter_context(tc.tile_pool(name="ep", bufs=1))

        # --- prologue: no GpSimd anywhere (its first custom op triggers a
        # multi-us ucode library load that would gate the masks).
        # tib (t_i replicated across partitions, read by every mask op) is
        # built with bf16 K=1 PE matmuls: psum = sum of the three bf16
        # components broadcast by ones[1,P].T @ tb_x — exact fp32 in psum
        # — then one ACT copy to SBUF.
        tb_rows = []
        for name, hnd in (("ta", tb_a), ("tb", tb_b), ("tc", tb_c)):
            row = const.tile([1, rows], BF16, tag=f"tbr_{name}")
            nc.sync.dma_start(row[:], hnd.ap().unsqueeze(0))
            tb_rows.append(row)

        # natural layout: t_pp[p, c] = t[p*ct + c]  (contiguous per
        # partition -> fast DMA); j-tile c is the stride-ct subset
        # {j : j % ct == c}, consistent across t_pp/w/wh tiles.
        t_pp = const.tile([P, ct], F32)
        nc.sync.dma_start(t_pp[:], t_all.ap().rearrange("(p c) -> p c", p=P))
        r_pp = const.tile([P, ct], F32)
        nc.sync.dma_start(r_pp[:], r_all.ap().rearrange("(p c) -> p c", p=P))
        r_t = ep.tile([P, ecols], F32)
        nc.sync.dma_start(r_t[:], r_blk.ap().rearrange("(p c) -> p c", c=ecols))

        ones_bf = const.tile([P, 1], BF16)
        nc.vector.memset(ones_bf[:], 1.0)
        ones_f = const.tile([P, 1], F32)
        nc.vector.memset(ones_f[:], 1.0)
        ones_row = const.tile([1, P], BF16)
        nc.vector.memset(ones_row[:], 1.0)

        tib_ps = psump.tile([P, rows], F32, tag="scratch")
        for s in range(nch):
            for k, row in enumerate(tb_rows):
                nc.tensor.matmul(
                    tib_ps[:, s * chunk : (s + 1) * chunk],
                    ones_row[:],
                    row[0:1, s * chunk : (s + 1) * chunk],
                    start=(k == 0), stop=(k == len(tb_rows) - 1),
                    skip_group_check=True,
                )
        tib = const.tile([P, rows], F32)
        nc.scalar.copy(tib[:], tib_ps[:])

        w_f = const.tile([P, ct], F32)
        nc.scalar.activation(w_f[:], r_pp[:], AFT.Exp)

        if have_act:
            mln2 = const.tile([P, 1], F32)
            nc.vector.memset(mln2[:], -0.6931471805599453)
            wh_bf = const.tile([P, ct], BF16)
            nc.scalar.activation(wh_bf[:], r_pp[:], AFT.Exp, bias=mln2[:])
            w_own = ep.tile([P, ecols], F32)
            nc.scalar.activation(w_own[:], r_t[:], AFT.Exp)
            tneg = const.tile([P, ct], F32)
            nc.vector.tensor_scalar(tneg[:], t_pp[:], -1.0, None, op0=ALU.mult)

        if have_act:
            # Whalf_tot = sum over ACT columns of wh (full f32): reduce to
            # [128,1], then two tiny N=1 matmuls: partition-sum -> [1,1]
            # -> broadcast back to [128,1]; lands in the Ln bias.  All of
            # it runs in the PE's early idle window.
            whsum = const.tile([P, 1], F32)
            parts = []
            for a, b, st in _act_slices(ct):
                pt = const.tile([P, 1], F32, tag=f"whp{a}")
                nc.vector.tensor_reduce(
                    pt[:], wh_bf[:, a:b:st], axis=AXL.X, op=ALU.add
                )
                parts.append(pt)
            if len(parts) == 1:
                nc.vector.tensor_copy(whsum[:], parts[0][:])
            else:
                nc.vector.tensor_add(whsum[:], parts[0][:], parts[1][:])
            ones_row_f = const.tile([1, P], F32)
            nc.vector.memset(ones_row_f[:], 1.0)
            ps1 = psump.tile([1, 1], F32, tag="scratch")
            nc.tensor.matmul(ps1[:], whsum[:], ones_f[:], start=True, stop=True)
            wtot1 = const.tile([1, 1], F32)
            nc.scalar.copy(wtot1[:], ps1[:])
            psb = psump.tile([P, 1], F32, tag="scratch")
            nc.tensor.matmul(psb[:], ones_row_f[:], wtot1[:], start=True, stop=True)
            ln_bias = const.tile([P, 1], F32)
            nc.vector.tensor_scalar(ln_bias[:], psb[:], EPS, None, op0=ALU.add)

        # --- main loop ---
        # DVE tiles accumulate into psum row 0 via a PE col-group-0
        # constant `ones` stationary; ACT tiles use col-group 1
        # (tile_position=(0,32) -> psum row 32) with per-tile wh columns
        # so the resident ones weights never alternate (weight-switch
        # costs ~2x per matmul when stationaries ping-pong).
        psum_rows = 33 if have_act else 1
        psum_t = psump.tile([psum_rows, rows], F32, tag="psum_t")
        v_tiles = [c for c in range(ct) if assign[c] == "v"]
        a_tiles = [c for c in range(ct) if assign[c] == "a"]

        for c in range(ct):
            m = masks.tile([P, rows], BF16, tag="mask")
            if assign[c] == "a":
                nc.scalar.activation(m[:], tib[:], AFT.Sign, bias=tneg[:, c : c + 1])
                lhsT = wh_bf[:, c : c + 1]
                prow, tpos = 32, (0, 32)
                start, stop = (c == a_tiles[0]), (c == a_tiles[-1])
            else:
                nc.vector.tensor_scalar(
                    m[:], tib[:], t_pp[:, c : c + 1], w_f[:, c : c + 1],
                    op0=ALU.is_ge, op1=ALU.mult,
                )
                lhsT = ones_bf[:]
                prow, tpos = 0, (0, 0)
                start, stop = (c == v_tiles[0]), (c == v_tiles[-1])
            for s in range(nch):
                nc.tensor.matmul(
                    psum_t[prow : prow + 1, s * chunk : (s + 1) * chunk],
                    lhsT,
                    m[:, s * chunk : (s + 1) * chunk],
                    start=start,
                    stop=stop,
                    tile_position=tpos,
                    skip_group_check=True,
                )

        # --- deferred plumbing (emitted after the main loop so its
        # DVE/DMA ops queue BEHIND the mask stream; the scheduler overlaps
        # them with the loop, none gate the masks) ---
        e_t = ep.tile([P, ecols], F32)
        nc.sync.dma_start(e_t[:], e_blk.ap().rearrange("(p c) -> p c", c=ecols))
        if not have_act:
            ln_bias = const.tile([P, 1], F32)
            nc.vector.memset(ln_bias[:], EPS)
        if have_act:
            dsel_t = ep.tile([P, ecols], F32)
            nc.sync.dma_start(dsel_t[:], dsel_b.ap().rearrange("(p c) -> p c", c=ecols))
            corr = ep.tile([P, ecols], F32)
            nc.vector.tensor_mul(corr[:], w_own[:], dsel_t[:])

        # --- epilogue ---
        # psum row(s) -> sbuf flat: chunked copies alternating ACT/DVE so
        # both engines drain the accumulators concurrently
        sefv = ep.tile([1, rows], F32)
        sefa = None
        if have_act:
            sefa = ep.tile([1, rows], F32, tag="sefa")
        half = rows // 2
        nc.scalar.copy(sefv[0:1, 0:half], psum_t[0:1, 0:half])
        nc.vector.tensor_copy(sefv[0:1, half:rows], psum_t[0:1, half:rows])
        sev = ep.tile([P, ecols], F32)
        nc.sync.dma_start(sev[:], sefv[0:1, :])
        if have_act:
            nc.scalar.copy(sefa[0:1, 0:half], psum_t[32:33, 0:half])
            nc.vector.tensor_copy(sefa[0:1, half:rows], psum_t[32:33, half:rows])
            sea = ep.tile([P, ecols], F32)
            nc.sync.dma_start(sea[:], sefa[0:1, :])
            se2 = ep.tile([P, ecols], F32)
            nc.vector.tensor_add(se2[:], sev[:], sea[:])
            nc.vector.tensor_add(se2[:], se2[:], corr[:])
        else:
            se2 = sev

        ln_t = ep.tile([P, ecols], F32)
        nc.scalar.activation(ln_t[:], se2[:], AFT.Ln, bias=ln_bias[:])
        d_t = ep.tile([P, ecols], F32)
        nc.vector.tensor_sub(d_t[:], r_t[:], ln_t[:])
        p_t = ep.tile([P, ecols], F32)
        nc.vector.tensor_mul(p_t[:], d_t[:], e_t[:])

        red = ep.tile([P, 2], F32)
        nc.vector.tensor_reduce(red[:, 0:1], p_t[:], axis=AXL.X, op=ALU.add)
        nc.vector.tensor_reduce(red[:, 1:2], e_t[:], axis=AXL.X, op=ALU.add)

        ps2 = psump.tile([2, 1], F32, tag="scratch")
        nc.tensor.matmul(ps2[:], red[:], ones_f[:], start=True, stop=True)
        out_sb = ep.tile([2, 1], F32)
        nc.scalar.copy(out_sb[:], ps2[:])
        nc.sync.dma_start(out_d.ap(), out_sb[:])

    nc.compile()
    return nc


_CACHE: dict = {}


def _get_nc():
    if "nc" not in _CACHE:
        _CACHE["nc"] = build()
    return _CACHE["nc"]


def make_dsel(n: int = N):
    # own column of row i lives in j-tile c = i % ct (natural layout)
    ct = n // P
    dsel = np.zeros(n, dtype=np.float32)
    acts = _act_set(ct)
    if acts:
        coltile = np.arange(n) % ct
        dsel[np.isin(coltile, sorted(acts))] = 0.5
    return dsel


def _bf16x3(x):
    # lossless fp32 -> (a, b, c) bf16 triple: a + b + c == x exactly
    import ml_dtypes

    a = x.astype(ml_dtypes.bfloat16)
    r1 = x - a.astype(np.float32)
    b = r1.astype(ml_dtypes.bfloat16)
    c = (r1 - b.astype(np.float32)).astype(ml_dtypes.bfloat16)
    return a, b, c


def make_in_maps(t, r, e, n=N, ncores=NCORES):
    rows = n // ncores
    dsel = make_dsel(n)
    in_maps = []
    for k in range(ncores):
        sl = slice(k * rows, (k + 1) * rows)
        ta, tb, tc = _bf16x3(np.ascontiguousarray(t[sl]))
        in_maps.append(
            {
                "t_all": t,
                "r_all": r,
                "tb_a": ta,
                "tb_b": tb,
                "tb_c": tc,
                "r_blk": np.ascontiguousarray(r[sl]),
                "e_blk": np.ascontiguousarray(e[sl]),
                "dsel_blk": np.ascontiguousarray(dsel[sl]),
            }
        )
    return in_maps


def combine(results, ncores=NCORES):
    ps = np.stack(
        [np.asarray(results[k]["out"], np.float64).reshape(2) for k in range(ncores)]
    )
    loss = -ps[:, 0].sum() / (ps[:, 1].sum() + EPS)
    return np.asarray(loss, dtype=np.float32)


def kernel(risk_scores, survival_time, event_indicator):
    r = np.ascontiguousarray(np.asarray(risk_scores, np.float32).reshape(-1))
    t = np.ascontiguousarray(np.asarray(survival_time, np.float32).reshape(-1))
    e = np.ascontiguousarray(np.asarray(event_indicator, np.float32).reshape(-1))
    assert r.shape == (N,) and t.shape == (N,) and e.shape == (N,)

    nc = _get_nc()
    res = bass_utils.run_bass_kernel_spmd(nc, make_in_maps(t, r, e), list(range(NCORES)))
    return combine(res.results)
